# revision 6
# baseline (speedup 1.0000x reference)
"""Trainium2 Bass kernel for nn_BidirectionalMLP (8-core SPMD).

Math (from the reference, EPS=0.5, BETA=0.5):
  states stay in [0,1] after every clipped update, so rho(s)=s for all
  state tensors; rx = clip(x,0,1) is fixed.
  Per relaxation step:
    s1' = clip(0.5*s1 + 0.25*(rx@fw0) + 0.25*(s2@bw1), 0, 1)
    s2' = clip(0.5*s2 + 0.25*(s1@fw1 + s3@bw2), 0, 1)
    s3' = clip(0.5*s3 + 0.5*(s2@fw2), 0, 1)              (free phase)
    s3' = clip(0.5*(s2@fw2) + 0.5*y, 0, 1)               (weak phase)
  20 free steps + 5 weak steps from zero states. Step 1 is degenerate:
  s1(1) = clip(0.25*(rx@fw0)), s2(1) = 0, s3(1) = 0 — computed in the
  preamble, so the main loop runs 24 iterations.

Sharding: output-feature sharding of the big weights. Core c owns columns
[512c, 512c+512) of fw1 and bw1 (and of bw2), keeping both resident in
SBUF as bf16 (4MB each). Each step all-gathers the bf16 states s1,s2
(two AllGathers of [512,256] per rank). Phases alternate order each
iteration so each AllGather has a full matmul phase to hide behind:
  phase A: psP1 = s2g@bw1 (+ ps3 = s2g@fw2) -> s1,s3 update -> AG(s1)
  phase B: psP2 = s1g@fw1 + s3@bw2          -> s2 update    -> AG(s2)
Matmuls are weight-stationary with transposed activations: out[feat,batch]
tiles [128,256], lhsT = weight chunk [128,128] bf16, rhs = gathered
activation chunk [128,256] bf16, fp32 PSUM accumulation over K.
"""

import numpy as np
import ml_dtypes

import concourse.bass as bass
import concourse.tile as tile
from concourse import bacc, mybir
from concourse.bass_utils import run_bass_kernel_spmd

N_CORES = 8
B = 256          # batch
D0 = 1024        # input dim
D = 4096         # hidden dims (layers 1 and 2)
D3 = 10          # output dim
F = D // N_CORES # 512 features per core per hidden layer
KC0 = D0 // 128  # 8
KC = D // 128    # 32
MC = F // 128    # 4
N_ITERS = 24     # steps 2..25 (step 1 done in preamble)
FREE_ITERS = 19  # iterations with free-phase s3 update (steps 2..20)

BF16 = mybir.dt.bfloat16
F32 = mybir.dt.float32
OP = mybir.AluOpType

_BUILD_CACHE: dict = {}


def _build(n_iters: int = N_ITERS, free_iters: int = FREE_ITERS):
    key = (n_iters, free_iters)
    if key in _BUILD_CACHE:
        return _BUILD_CACHE[key]

    nc = bacc.Bacc("TRN2", target_bir_lowering=False, debug=False,
                   num_devices=N_CORES, enable_asserts=False)

    # --- per-core external I/O (weights pre-arranged host-side to SBUF layout) ---
    fw0c = nc.dram_tensor("fw0c", [128, KC0 * F], BF16, kind="ExternalInput")
    fw1c = nc.dram_tensor("fw1c", [128, KC * F], BF16, kind="ExternalInput")
    bw1c = nc.dram_tensor("bw1c", [128, KC * F], BF16, kind="ExternalInput")
    fw2r = nc.dram_tensor("fw2r", [128, KC * D3], BF16, kind="ExternalInput")
    bw2c = nc.dram_tensor("bw2c", [D3, F], BF16, kind="ExternalInput")
    rxT = nc.dram_tensor("rxT", [128, KC0 * B], BF16, kind="ExternalInput")
    yh = nc.dram_tensor("yh", [D3, B], F32, kind="ExternalInput")
    o1 = nc.dram_tensor("o1", [F, B], F32, kind="ExternalOutput")
    o2 = nc.dram_tensor("o2", [F, B], F32, kind="ExternalOutput")
    o3 = nc.dram_tensor("o3", [D3, B], F32, kind="ExternalOutput")

    with tile.TileContext(nc) as tc:
        with tc.tile_pool(name="wp", bufs=1) as wp, \
             tc.tile_pool(name="st", bufs=1) as st, \
             tc.tile_pool(name="wk", bufs=2) as wk, \
             tc.tile_pool(name="gp", bufs=2) as gp, \
             tc.tile_pool(name="pp", bufs=1, space="PSUM") as pp, \
             tc.tile_pool(name="dp", bufs=2, space="DRAM") as dp:

            # ---- load weights / constants ----
            w_fw1 = wp.tile([128, KC * F], BF16)
            nc.sync.dma_start(w_fw1[:], fw1c[:])
            w_bw1 = wp.tile([128, KC * F], BF16)
            nc.sync.dma_start(w_bw1[:], bw1c[:])
            w_fw0 = wp.tile([128, KC0 * F], BF16)
            nc.sync.dma_start(w_fw0[:], fw0c[:])
            w_fw2 = wp.tile([128, KC * D3], BF16)
            nc.sync.dma_start(w_fw2[:], fw2r[:])
            w_bw2 = wp.tile([D3, F], BF16)
            nc.sync.dma_start(w_bw2[:], bw2c[:])
            t_rx = wp.tile([128, KC0 * B], BF16)
            nc.sync.dma_start(t_rx[:], rxT[:])
            t_yh = wp.tile([D3, B], F32)
            nc.sync.dma_start(t_yh[:], yh[:])

            # ---- persistent fp32 state ----
            s1 = st.tile([128, MC * B], F32)
            s2 = st.tile([128, MC * B], F32)
            s3 = st.tile([D3, B], F32)
            cc = st.tile([128, MC * B], F32)   # 0.25*(rx@fw0) slice

            # ---- preamble: c1 and step-1 states ----
            for m in range(MC):
                psc = pp.tile([128, B], F32, tag="psc")
                for k in range(KC0):
                    nc.tensor.matmul(
                        psc[:],
                        w_fw0[:, k * F + m * 128: k * F + (m + 1) * 128],
                        t_rx[:, k * B:(k + 1) * B],
                        start=(k == 0), stop=(k == KC0 - 1))
                sm = slice(m * B, (m + 1) * B)
                nc.vector.tensor_scalar_mul(cc[:, sm], psc[:], 0.25)
                nc.vector.tensor_scalar(s1[:, sm], cc[:, sm], 0.0, 1.0,
                                        OP.max, OP.min)
            nc.vector.memset(s2[:], 0.0)
            nc.vector.memset(s3[:], 0.0)
            s3b = wk.tile([D3, B], BF16, tag="s3b")
            nc.vector.memset(s3b[:], 0.0)

            def emit_ag(src_f32, which: str, zero: bool = False):
                """bf16-stage src, AllGather it, return gathered SBUF tile
                [128, KC*B] with chunk j at columns [j*B, (j+1)*B)."""
                sg = wk.tile([128, MC * B], BF16, tag=f"sg{which}",
                             name=f"sg{which}")
                if zero:
                    nc.vector.memset(sg[:], 0.0)
                else:
                    for m in range(MC):
                        sm = slice(m * B, (m + 1) * B)
                        nc.scalar.copy(sg[:, sm], src_f32[:, sm])
                agin = dp.tile([F, B], BF16, tag=f"agin{which}",
                               name=f"agin{which}")
                nc.sync.dma_start(
                    agin.rearrange("(j p) b -> p j b", p=128),
                    sg[:].rearrange("p (j b) -> p j b", b=B))
                agout = dp.tile([D, B], BF16, tag=f"agout{which}",
                                name=f"agout{which}", addr_space="Shared")
                nc.gpsimd.collective_compute(
                    "AllGather", OP.bypass,
                    replica_groups=[list(range(N_CORES))],
                    ins=[agin.opt()], outs=[agout.opt()])
                g = gp.tile([128, KC * B], BF16, tag=f"g{which}",
                            name=f"g{which}")
                ago = agout.rearrange("(j p) b -> p j b", p=128)
                g3d = g[:].rearrange("p (j b) -> p j b", b=B)
                for q in range(4):
                    nc.sync.dma_start(g3d[:, q * 8:(q + 1) * 8, :],
                                      ago[:, q * 8:(q + 1) * 8, :])
                return g

            # initial gathers for iteration 0 (s2 gather is all zeros;
            # emitted first because phase A consumes it first)
            g2_cur = emit_ag(None, "2", zero=True)
            g1_cur = emit_ag(s1, "1")

            def phase_a(g2, weak: bool, last: bool):
                """P1 = s2g@bw1 (+P3 = s2g@fw2); update s1, s3; AG(s1)."""
                # one PSUM bank per accumulation group (two interleaved
                # groups in one bank corrupt each other)
                pst = [pp.tile([128, B], F32, tag=f"mm{m}", name=f"pa{m}")[:]
                       for m in range(MC)]
                p3 = pp.tile([D3, B], F32, tag="p3", name="p3")
                for j in range(KC):
                    rhs = g2[:, j * B:(j + 1) * B]
                    for m in range(MC):
                        nc.tensor.matmul(
                            pst[m],
                            w_bw1[:, j * F + m * 128: j * F + (m + 1) * 128],
                            rhs, start=(j == 0), stop=(j == KC - 1))
                    nc.tensor.matmul(p3[:],
                                     w_fw2[:, j * D3:(j + 1) * D3],
                                     rhs, start=(j == 0), stop=(j == KC - 1))
                # s1 <- clip(0.25*psP1 + 0.5*s1 + cc)
                for m in range(MC):
                    sm = slice(m * B, (m + 1) * B)
                    t1 = wk.tile([128, B], F32, tag="t1", name="t1")
                    nc.vector.scalar_tensor_tensor(
                        t1[:], pst[m], 0.5, s1[:, sm], OP.mult, OP.add)
                    t2 = wk.tile([128, B], F32, tag="t2", name="t2")
                    nc.vector.scalar_tensor_tensor(
                        t2[:], t1[:], 0.5, cc[:, sm], OP.mult, OP.add)
                    nc.vector.tensor_scalar(s1[:, sm], t2[:], 0.0, 1.0,
                                            OP.max, OP.min)
                # s3 update
                if weak:
                    u3 = wk.tile([D3, B], F32, tag="u3", name="u3")
                    nc.vector.scalar_tensor_tensor(
                        u3[:], p3[:], 0.5, t_yh[:], OP.mult, OP.add)
                    nc.vector.tensor_scalar(s3[:], u3[:], 0.0, 1.0,
                                            OP.max, OP.min)
                else:
                    u3 = wk.tile([D3, B], F32, tag="u3", name="u3")
                    nc.vector.tensor_tensor(u3[:], p3[:], s3[:], OP.add)
                    v3 = wk.tile([D3, B], F32, tag="v3", name="v3")
                    nc.vector.tensor_scalar(v3[:], u3[:], 0.5, 0.0,
                                            OP.mult, OP.max)
                    nc.vector.tensor_scalar_min(s3[:], v3[:], 1.0)
                if last:
                    return None, None
                s3b_next = wk.tile([D3, B], BF16, tag="s3b", name="s3b")
                nc.scalar.copy(s3b_next[:], s3[:])
                g1_next = emit_ag(s1, "1")
                return g1_next, s3b_next

            def phase_b(g1, s3b_cur, last: bool):
                """P2 = s1g@fw1 + s3@bw2; update s2; AG(s2)."""
                pst = [pp.tile([128, B], F32, tag=f"mm{m}", name=f"pb{m}")[:]
                       for m in range(MC)]
                for j in range(KC):
                    rhs = g1[:, j * B:(j + 1) * B]
                    for m in range(MC):
                        nc.tensor.matmul(
                            pst[m],
                            w_fw1[:, j * F + m * 128: j * F + (m + 1) * 128],
                            rhs, start=(j == 0), stop=False)
                for m in range(MC):
                    nc.tensor.matmul(pst[m],
                                     w_bw2[:, m * 128:(m + 1) * 128],
                                     s3b_cur[:], start=False, stop=True)
                # s2 <- clip(0.25*psP2 + 0.5*s2)
                for m in range(MC):
                    sm = slice(m * B, (m + 1) * B)
                    t1 = wk.tile([128, B], F32, tag="t1", name="t1")
                    nc.vector.scalar_tensor_tensor(
                        t1[:], pst[m], 0.5, s2[:, sm], OP.mult, OP.add)
                    t2 = wk.tile([128, B], F32, tag="t2", name="t2")
                    nc.vector.tensor_scalar(t2[:], t1[:], 0.5, 0.0,
                                            OP.mult, OP.max)
                    nc.vector.tensor_scalar_min(s2[:, sm], t2[:], 1.0)
                if last:
                    return None
                return emit_ag(s2, "2")

            for t in range(n_iters):
                weak = t >= free_iters
                last = t == n_iters - 1
                if t % 2 == 0:
                    g1_next, s3b_next = phase_a(g2_cur, weak, last)
                    g2_next = phase_b(g1_cur, s3b, last)
                else:
                    g2_next = phase_b(g1_cur, s3b, last)
                    g1_next, s3b_next = phase_a(g2_cur, weak, last)
                g1_cur, g2_cur, s3b = g1_next, g2_next, s3b_next

            # ---- outputs ----
            nc.sync.dma_start(o1.ap().rearrange("(j p) b -> p j b", p=128),
                              s1[:].rearrange("p (j b) -> p j b", b=B))
            nc.sync.dma_start(o2.ap().rearrange("(j p) b -> p j b", p=128),
                              s2[:].rearrange("p (j b) -> p j b", b=B))
            nc.sync.dma_start(o3.ap(), s3[:])

    nc.compile()
    _BUILD_CACHE[key] = nc
    return nc


def _rearr_w(w: np.ndarray, kc: int) -> np.ndarray:
    """[kc*128, M] -> [128, kc*M] with chunk k at cols [k*M,(k+1)*M)."""
    n, m = w.shape
    assert n == kc * 128
    return np.ascontiguousarray(
        w.reshape(kc, 128, m).transpose(1, 0, 2).reshape(128, kc * m))


def _prep_in_maps(x, fw0, fw1, fw2, bw1, bw2, y_one_hot):
    bf = ml_dtypes.bfloat16
    x = np.asarray(x, np.float32)
    rxT = np.clip(x, 0.0, 1.0).T.astype(np.float32)        # [1024, 256]
    rxT_r = _rearr_w(rxT, KC0).astype(bf)                   # [128, 8*256]
    fw2_r = _rearr_w(np.asarray(fw2, np.float32), KC).astype(bf)
    yh = (0.5 * np.asarray(y_one_hot, np.float32).T).astype(np.float32)
    yh = np.ascontiguousarray(yh)
    in_maps = []
    for c in range(N_CORES):
        sl = slice(c * F, (c + 1) * F)
        in_maps.append({
            "fw0c": _rearr_w(np.asarray(fw0, np.float32)[:, sl], KC0).astype(bf),
            "fw1c": _rearr_w(np.asarray(fw1, np.float32)[:, sl], KC).astype(bf),
            "bw1c": _rearr_w(np.asarray(bw1, np.float32)[:, sl], KC).astype(bf),
            "fw2r": fw2_r,
            "bw2c": np.ascontiguousarray(np.asarray(bw2, np.float32)[:, sl]).astype(bf),
            "rxT": rxT_r,
            "yh": yh,
        })
    return in_maps


def _assemble(results) -> np.ndarray:
    s1 = np.concatenate([results[c]["o1"] for c in range(N_CORES)], axis=0).T
    s2 = np.concatenate([results[c]["o2"] for c in range(N_CORES)], axis=0).T
    s3 = results[0]["o3"].T
    return np.ascontiguousarray(
        np.concatenate([s1, s2, s3], axis=1).astype(np.float32))


def run(inputs: dict, trace: bool = False, n_iters: int = N_ITERS,
        free_iters: int = FREE_ITERS):
    """Returns (output [256, 8202] fp32, BassKernelResults)."""
    nc = _build(n_iters, free_iters)
    in_maps = _prep_in_maps(
        inputs["x"], inputs["fw0"], inputs["fw1"], inputs["fw2"],
        inputs["bw1"], inputs["bw2"], inputs["y_one_hot"])
    r = run_bass_kernel_spmd(nc, in_maps, core_ids=list(range(N_CORES)),
                             trace=trace)
    return _assemble(r.results), r


def kernel(**inputs) -> np.ndarray:
    out, _ = run(inputs)
    return out
